# revision 1
# baseline (speedup 1.0000x reference)
"""Trainium2 Bass kernel for the differentiable isotropic-Gaussian renderer.

Math: the reference computes, per batch b,
    w[n, pix] = opac_n * exp(-0.5 * ||c_pix - proj_n||^2 / scales_n^2)
    out[c]    = (w.T @ colors) / (w.sum(0) + EPS)
Each gaussian is isotropic and the pixel grid is separable
(pix = (x, y), x in 0..W-1, y in 0..H-1), so the weight factorizes:
    w[n, (y,x)] = opac_n * Ey[n, y] * Ex[n, x]
    Ex[n, x] = exp(-((x - mx_n) * sqrt(.5)/s_n)^2),  Ey likewise.
The render collapses to 4 matmuls per image,
    S_j[y, x] = sum_n (q_j[n] * Ey[n, y]) * Ex[n, x],
with q_0 = opac (denominator), q_{1..3} = opac * color_c, then
out[c] = S_{c+1} / (S_0 + EPS).  This replaces the N x H*W dense weight
matrix (67M exps/batch) with N*(H+W) exps (0.5M/batch).

Sharding: 8 cores = 2 batches x 4 y-quarters (64 rows each).  Every core
gets the full gaussian set (replicated; tiny) plus a per-core projection
matrix and y-grid, computes its [3, 64, 256] slice entirely locally (no
collectives), and the host reassembles the [2, 3, 256, 256] output.

Perf notes (from the instruction-cost timeline sim):
 - render matmuls run in exact fp32 (RENDER_MODE="fp32"); channel pairs
   are stacked along the output partition dim (den duplicated as a 5th
   weight channel) so M=128 and only 24 matmuls stream instead of 32.
   RENDER_MODE="f32r" uses the PE's single-pass reduced-precision mode
   (~5 us faster end-to-end, ~8e-4 max rel err vs the reference).
 - inputs are host-packed into two contiguous tensors + a 13-float aux
   row; pixel grids are generated on-device (iota + per-core y offset)
 - dummy PE matmuls between the projection and the renders keep the
   tensor engine's HAM p-state at full clock
 - engine balance: PE projection/renders, ACT exps (y first: it gates
   the lhsT path), DVE fused affine+squares+epilogue, GPSIMD q*Ey
"""

import numpy as np

import concourse.bacc as bacc
import bass_rust
import concourse.bass as bass
import concourse.tile as tile
from concourse import mybir
from concourse.bass_utils import run_bass_kernel_spmd

H, W = 256, 256
FX, FY = 300.0, 300.0
CX, CY = 128.0, 128.0
N = 1024
B = 2
EPS = 1e-8
NCORES = 8
YQ = H // 4          # y-rows per core
NCHUNK = N // 128    # gaussian partition chunks

TRACE = False
LAST_RESULTS = None
_CACHED_NC = None
# render matmul mode: "fp32" = bit-exact fp32 (2-pass PE matmuls, channel
# pairs stacked to M=128); "f32r" = single-pass reduced-precision mode
# (~4x faster PE, ~8e-4 max rel err vs the reference)
RENDER_MODE = "fp32"


def build_kernel(nc, sb, ps):
    f32 = mybir.dt.float32
    f32r = mybir.dt.float32r
    AT = mybir.AluOpType
    AF = mybir.ActivationFunctionType

    posT = nc.dram_tensor("post", [4, N], f32, kind="ExternalInput")
    gprops = nc.dram_tensor("gprops", [128, NCHUNK, 5], f32, kind="ExternalInput")
    # aux row: pm (12 floats) + y0 (1 float, the core's y-quarter offset)
    aux = nc.dram_tensor("aux", [1, 13], f32, kind="ExternalInput")
    out = nc.dram_tensor("out", [3, YQ, W], f32, kind="ExternalOutput")

    # ---------------- input loads ----------------
    posTs = sb.tile([4, N], f32, tag="posTs")
    nc.sync.dma_start(out=posTs[:, :], in_=posT[:, :])
    pmt = sb.tile([4, 3], f32, tag="pmt")
    nc.sync.dma_start(
        out=pmt[:, :],
        in_=bass.AP(tensor=aux, offset=0, ap=[[3, 4], [1, 3]]),
    )
    gp = sb.tile([128, NCHUNK, 5], f32, tag="gp")
    nc.scalar.dma_start(out=gp[:, :, :], in_=gprops[:, :, :])
    grow = sb.tile([1, 1], f32, tag="grow")
    nc.scalar.dma_start(
        out=grow[:, :],
        in_=bass.AP(tensor=aux, offset=12, ap=[[0, 1], [1, 1]]),
    )

    # pixel grids generated on-device at t=0 (x is 0..W-1 on every core;
    # y is 0..YQ-1 plus the per-core offset y0 from aux)
    xg_i = sb.tile([128, W], mybir.dt.int32, tag="xg_i")
    nc.gpsimd.iota(xg_i[:, :], pattern=[[1, W]], base=0, channel_multiplier=0)
    xg = sb.tile([128, W], f32, tag="xg")
    nc.gpsimd.tensor_copy(xg[:, :], xg_i[:, :])
    yg_i = sb.tile([128, YQ], mybir.dt.int32, tag="yg_i")
    nc.gpsimd.iota(yg_i[:, :], pattern=[[1, YQ]], base=0, channel_multiplier=0)
    yg0 = sb.tile([128, YQ], f32, tag="yg0")
    nc.gpsimd.tensor_copy(yg0[:, :], yg_i[:, :])
    y0b = sb.tile([128, 1], f32, tag="y0b")
    nc.gpsimd.partition_broadcast(y0b[:, :], grow[:, :])
    yg = sb.tile([128, YQ], f32, tag="yg")
    nc.gpsimd.tensor_scalar_add(yg[:, :], yg0[:, :], y0b[:, :])

    # ---------------- projection (PE) ----------------
    uvz_ps = ps.tile([128, NCHUNK * 3], f32, tag="uvz_ps")
    for c in range(NCHUNK):
        nc.tensor.matmul(
            uvz_ps[:, c * 3 : (c + 1) * 3],
            lhsT=posTs[:, c * 128 : (c + 1) * 128],
            rhs=pmt[:, :],
            start=True,
            stop=True,
        )
    uvz = uvz_ps.rearrange("p (c k) -> p c k", k=3)

    # keep the PE p-state warm between the projection and render matmuls
    # (otherwise the renders start at the throttled clock); results unused
    warm_ps = ps.tile([128, 112], f32, tag="warm_ps")
    for i in range(20):
        nc.tensor.matmul(
            warm_ps[:, :],
            lhsT=posTs[:, 0:128],
            rhs=posTs[:, 0:112],
            start=True,
            stop=True,
        )

    # ---------------- per-gaussian prep (DVE) ----------------
    # sp first: it only needs the gp DMA (arrives before the projection),
    # so it clears the DVE queue before the proj-dependent ops
    # sp = 1 / scale -> exponent = -0.5 * ((x - mx) * sp)^2 (0.5 folded
    # into the Exp scale)
    sp = sb.tile([128, NCHUNK], f32, tag="sp")
    nc.vector.reciprocal(sp[:, :], gp[:, :, 0])
    rz = sb.tile([128, NCHUNK], f32, tag="rz")
    nc.vector.reciprocal(rz[:, :], uvz[:, :, 2])
    my = sb.tile([128, NCHUNK], f32, tag="my")
    nc.vector.tensor_mul(my[:, :], uvz[:, :, 1], rz[:, :])
    mx = sb.tile([128, NCHUNK], f32, tag="mx")
    nc.vector.tensor_mul(mx[:, :], uvz[:, :, 0], rz[:, :])

    # ---------------- separable factors ----------------
    # y-side first: it feeds the matmul lhsT path (exp -> q*Ey on GPSIMD)
    ty = sb.tile([128, NCHUNK, YQ], f32, tag="ty")
    for c in range(NCHUNK):
        nc.vector.tensor_scalar(
            ty[:, c, :], yg[:, :], my[:, c : c + 1], sp[:, c : c + 1],
            op0=AT.subtract, op1=AT.mult,
        )
    ey = sb.tile([128, NCHUNK, YQ], f32, tag="ey")
    expy_is = []
    for h in range(2):
        hc = NCHUNK // 2
        yflat = ty[:, h * hc : (h + 1) * hc, :].rearrange("p c y -> p (c y)")
        eyflat = ey[:, h * hc : (h + 1) * hc, :].rearrange("p c y -> p (c y)")
        nc.scalar.activation(eyflat, yflat, AF.Square)
        expy_is.append(nc.scalar.activation(eyflat, eyflat, AF.Exp, scale=-0.5))
    expy_i = expy_is[-1]

    # x-side: fused affine on DVE, square on DVE (keeps ACT = exps only),
    # exp on ACT in groups so the PE can start on the first chunks
    tx = sb.tile([128, NCHUNK, W], f32, tag="tx")
    mm_dt = f32r if RENDER_MODE == "f32r" else f32
    ex = sb.tile([128, NCHUNK, W], mm_dt, tag="ex")
    for c0, c1 in ((0, 2), (2, 4), (4, 6), (6, 8)):
        for c in range(c0, c1):
            nc.vector.tensor_scalar(
                tx[:, c, :], xg[:, :], mx[:, c : c + 1], sp[:, c : c + 1],
                op0=AT.subtract, op1=AT.mult,
            )
        txh = tx[:, c0:c1, :].rearrange("p c x -> p (c x)")
        exh = ex[:, c0:c1, :].rearrange("p c x -> p (c x)")
        nc.vector.tensor_mul(exh, txh, txh)
        expx_i = nc.scalar.activation(exh, exh, AF.Exp, scale=-0.5)
        bass_rust.add_dep_helper(expx_i.ins, expy_i.ins, sync=False,
                                 reason="Exp_y feeds the PE-critical lhsT path")

    # channel weights (emitted late: only needed by wmat).
    # f32r: [opac, opac*r, opac*g, opac*b]
    # fp32: [opac, opac, opac*r, opac*g, opac*b] (den twice so channel
    # pairs stack to M=128 with den available on both partition halves)
    NQ = 5
    qw = sb.tile([128, NCHUNK, NQ], f32, tag="qw")
    for j in range(NQ - 3):
        nc.gpsimd.tensor_copy(qw[:, :, j], gp[:, :, 1])
    for ch in range(3):
        nc.gpsimd.tensor_mul(
            qw[:, :, NQ - 3 + ch], gp[:, :, 1], gp[:, :, ch + 2]
        )

    # ---------------- channel-scaled Ey (GPSIMD) ----------------
    # wmat[:, c, j, :] = qw[:, c, j] * ey[:, c, :]  via broadcast reads
    wmat = sb.tile([128, NCHUNK, NQ, YQ], mm_dt, tag="wmat")

    def wmat_op(eng, c0, c1):
        eyc = ey[:, c0:c1, :]
        ey_b = bass.AP(
            tensor=ey.tensor, offset=eyc.offset,
            ap=[eyc.ap[0], eyc.ap[1], [0, NQ], eyc.ap[2]],
        )
        qwc = qw[:, c0:c1, :]
        q_b = bass.AP(
            tensor=qw.tensor, offset=qwc.offset,
            ap=[qwc.ap[0], qwc.ap[1], qwc.ap[2], [0, YQ]],
        )
        eng.tensor_mul(wmat[:, c0:c1, :, :], ey_b, q_b)

    wmat_op(nc.gpsimd, 0, 1)
    wmat_op(nc.gpsimd, 1, 3)
    wmat_op(nc.gpsimd, 3, 5)
    wmat_op(nc.vector, 5, 8)

    # ---------------- render matmuls (PE, float32r) ----------------
    # ---------------- render matmuls + epilogue ----------------
    # Channel pairs stacked along the output partition dim (M=128) to
    # halve the streamed column count: acc_dd = [den | den], acc_rg =
    # [red | grn], acc_b = [blu] (M=64).  fp32 runs the PE's exact 2-pass
    # mode; f32r runs single-pass (~4x faster, ~8e-4 max rel err).
    acc_dd = ps.tile([128, W], f32, tag="acc_dd")
    acc_rg = ps.tile([128, W], f32, tag="acc_rg")
    acc_b = ps.tile([YQ, W], f32, tag="acc_b")
    # (chain, chunk) issue order: interleaved per chunk so each ex[c] is
    # consumed as it lands, EXCEPT the last two den matmuls are hoisted
    # ahead of the last color matmuls so the eps+reciprocal chain overlaps
    # the tail of the render stream.
    sched = [(c, j) for c in range(NCHUNK - 2) for j in range(3)]
    sched += [(6, 0), (7, 0), (6, 1), (7, 1), (6, 2), (7, 2)]
    chain_args = [
        lambda c: dict(lhsT=wmat[:, c, 0:2, :], out=acc_dd[:, :]),
        lambda c: dict(lhsT=wmat[:, c, 2:4, :], out=acc_rg[:, :]),
        lambda c: dict(lhsT=wmat[:, c, 4, :], out=acc_b[:, :]),
    ]
    for c, j in sched:
        a = chain_args[j](c)
        nc.tensor.matmul(
            a["out"], lhsT=a["lhsT"], rhs=ex[:, c, :],
            start=(c == 0), stop=(c == NCHUNK - 1),
        )
    rden = sb.tile([128, W], f32, tag="rden")
    nc.vector.tensor_scalar_add(rden[:, :], acc_dd[:, :], EPS)
    nc.vector.reciprocal(rden[:, :], rden[:, :])
    outt_rg = sb.tile([128, W], f32, tag="outt_rg")
    nc.vector.tensor_mul(outt_rg[:, :], acc_rg[:, :], rden[:, :])
    outt_b = sb.tile([YQ, W], f32, tag="outt_b")
    nc.vector.tensor_mul(outt_b[:, :], acc_b[:, :], rden[0:YQ, :])
    # rg on the ACT HWDGE queue, b (the critical tail transfer: it
    # follows the very last matmul) on the SP queue whose DMA-start
    # delay is 134ns shorter
    nc.scalar.dma_start(
        out=out[0:2, :, :].rearrange("c y x -> (c y) x"), in_=outt_rg[:, :]
    )
    nc.sync.dma_start(out=out[2, :, :], in_=outt_b[:, :])


def _build_module():
    nc = bacc.Bacc("TRN2", target_bir_lowering=False, debug=False)
    with tile.TileContext(nc) as tc:
        with (
            tc.tile_pool(name="sb", bufs=1) as sb,
            tc.tile_pool(name="ps", bufs=1, space="PSUM") as ps,
        ):
            build_kernel(nc, sb, ps)
    nc.compile()
    return nc


def _host_pm(qvec_b: np.ndarray, tvec_b: np.ndarray) -> np.ndarray:
    """Combined projection matrix: [x y z 1] @ pm = (u, v, z_cam) with
    proj = (u/z_cam, v/z_cam).  Mirrors reference._quat_to_rot."""
    q = qvec_b.astype(np.float64)
    q = q / np.linalg.norm(q)
    w_, x, y, z = q
    R = np.array(
        [
            [1 - 2 * (y * y + z * z), 2 * (x * y - z * w_), 2 * (x * z + y * w_)],
            [2 * (x * y + z * w_), 1 - 2 * (x * x + z * z), 2 * (y * z - x * w_)],
            [2 * (x * z - y * w_), 2 * (y * z + x * w_), 1 - 2 * (x * x + y * y)],
        ]
    )
    t = tvec_b.astype(np.float64)
    rows = np.stack([FX * R[0] + CX * R[2], FY * R[1] + CY * R[2], R[2]], axis=1)
    last = np.array([FX * t[0] + CX * t[2], FY * t[1] + CY * t[2], t[2]])
    return np.concatenate([rows, last[None, :]], axis=0).astype(np.float32)


def kernel(positions, colors, opacities, scales, qvec, tvec, pixel_coords):
    global _CACHED_NC, LAST_RESULTS
    if _CACHED_NC is None:
        _CACHED_NC = _build_module()
    nc = _CACHED_NC

    f32 = np.float32
    pos = np.asarray(positions, f32)
    colv = np.asarray(colors, f32)
    opv = np.asarray(opacities, f32).reshape(N)
    scv = np.asarray(scales, f32).reshape(N)
    pc = np.asarray(pixel_coords, f32).reshape(H, W, 2)
    xs = np.ascontiguousarray(pc[0, :, 0].reshape(1, W))
    ys_full = pc[:, 0, 1]

    # host-side packing (layout only; all math stays on device)
    posT_h = np.ascontiguousarray(
        np.concatenate([pos.T, np.ones((1, N), f32)], axis=0)
    )  # [4, N]
    gprops_h = np.ascontiguousarray(
        np.concatenate([scv.reshape(N, 1), opv.reshape(N, 1), colv], axis=1)
        .reshape(NCHUNK, 128, 5)
        .transpose(1, 0, 2)
    )  # [128, NCHUNK, 5]

    pms = [
        _host_pm(np.asarray(qvec, f32)[b], np.asarray(tvec, f32)[b]) for b in range(B)
    ]

    in_maps = []
    for core in range(NCORES):
        b, qy = divmod(core, 4)
        aux = np.concatenate(
            [pms[b].reshape(-1), ys_full[qy * YQ : qy * YQ + 1]]
        ).reshape(1, 13)
        in_maps.append(
            dict(
                post=posT_h,
                gprops=gprops_h,
                aux=np.ascontiguousarray(aux, dtype=f32),
            )
        )

    def _run_and_gather():
        res = run_bass_kernel_spmd(
            nc, in_maps, core_ids=list(range(NCORES)), trace=TRACE
        )
        outv = np.zeros((B, 3, H, W), f32)
        for core in range(NCORES):
            b, qy = divmod(core, 4)
            outv[b, :, qy * YQ : (qy + 1) * YQ, :] = np.asarray(
                res.results[core]["out"]
            )
        return res, outv

    # retries: the axon-proxied execute occasionally fails with a
    # transient NRT_EXEC_UNIT_UNRECOVERABLE worker error (can surface
    # lazily at result-fetch time) that clears on a later attempt
    last_exc = None
    for _attempt in range(3):
        try:
            res, outv = _run_and_gather()
            break
        except Exception as e:  # noqa: BLE001
            last_exc = e
    else:
        raise last_exc
    LAST_RESULTS = res
    return outv



# revision 2
# speedup vs baseline: 1.2606x; 1.2606x over previous
"""Trainium2 Bass kernel for the differentiable isotropic-Gaussian renderer.

Math: the reference computes, per batch b,
    w[n, pix] = opac_n * exp(-0.5 * ||c_pix - proj_n||^2 / scales_n^2)
    out[c]    = (w.T @ colors) / (w.sum(0) + EPS)
Each gaussian is isotropic and the pixel grid is separable
(pix = (x, y), x in 0..W-1, y in 0..H-1), so the weight factorizes:
    w[n, (y,x)] = opac_n * Ey[n, y] * Ex[n, x]
    Ex[n, x] = exp(-((x - mx_n) * sqrt(.5)/s_n)^2),  Ey likewise.
The render collapses to 4 matmuls per image,
    S_j[y, x] = sum_n (q_j[n] * Ey[n, y]) * Ex[n, x],
with q_0 = opac (denominator), q_{1..3} = opac * color_c, then
out[c] = S_{c+1} / (S_0 + EPS).  This replaces the N x H*W dense weight
matrix (67M exps/batch) with N*(H+W) exps (0.5M/batch).

Sharding: 8 cores = 2 batches x 4 y-quarters (64 rows each).  Every core
gets the full gaussian set (replicated; tiny) plus a per-core projection
matrix and y-grid, computes its [3, 64, 256] slice entirely locally (no
collectives), and the host reassembles the [2, 3, 256, 256] output.

Perf notes (from the instruction-cost timeline sim):
 - render matmuls run in exact fp32 (RENDER_MODE="fp32"); channel pairs
   are stacked along the output partition dim (den duplicated as a 5th
   weight channel) so M=128 and only 24 matmuls stream instead of 32.
   RENDER_MODE="f32r" uses the PE's single-pass reduced-precision mode
   (~5 us faster end-to-end, ~8e-4 max rel err vs the reference).
 - inputs are host-packed into two contiguous tensors + a 13-float aux
   row; pixel grids are generated on-device (iota + per-core y offset)
 - dummy PE matmuls between the projection and the renders keep the
   tensor engine's HAM p-state at full clock
 - engine balance: PE projection/renders, ACT exps (y first: it gates
   the lhsT path), DVE fused affine+squares+epilogue, GPSIMD q*Ey
"""

import numpy as np

import concourse.bacc as bacc
import bass_rust
import concourse.bass as bass
import concourse.tile as tile
from concourse import mybir
from concourse.bass_utils import run_bass_kernel_spmd

H, W = 256, 256
FX, FY = 300.0, 300.0
CX, CY = 128.0, 128.0
N = 1024
B = 2
EPS = 1e-8
NCORES = 8
YQ = H // 4          # y-rows per core
NCHUNK = N // 128    # gaussian partition chunks

TRACE = False
LAST_RESULTS = None
_CACHED_NC = None
# render matmul mode: "fp32" = bit-exact fp32 (2-pass PE matmuls, channel
# pairs stacked to M=128); "f32r" = single-pass reduced-precision mode
# (~4x faster PE, ~8e-4 max rel err vs the reference)
RENDER_MODE = "f32r"


def build_kernel(nc, sb, ps):
    f32 = mybir.dt.float32
    f32r = mybir.dt.float32r
    AT = mybir.AluOpType
    AF = mybir.ActivationFunctionType

    posT = nc.dram_tensor("post", [4, N], f32, kind="ExternalInput")
    gprops = nc.dram_tensor("gprops", [128, NCHUNK, 5], f32, kind="ExternalInput")
    # aux row: pm (12 floats) + y0 (1 float, the core's y-quarter offset)
    aux = nc.dram_tensor("aux", [1, 13], f32, kind="ExternalInput")
    out = nc.dram_tensor("out", [3, YQ, W], f32, kind="ExternalOutput")

    # ---------------- input loads ----------------
    posTs = sb.tile([4, N], f32, tag="posTs")
    nc.sync.dma_start(out=posTs[:, :], in_=posT[:, :])
    pmt = sb.tile([4, 3], f32, tag="pmt")
    nc.sync.dma_start(
        out=pmt[:, :],
        in_=bass.AP(tensor=aux, offset=0, ap=[[3, 4], [1, 3]]),
    )
    gp = sb.tile([128, NCHUNK, 5], f32, tag="gp")
    nc.scalar.dma_start(out=gp[:, :, :], in_=gprops[:, :, :])
    grow = sb.tile([1, 1], f32, tag="grow")
    nc.scalar.dma_start(
        out=grow[:, :],
        in_=bass.AP(tensor=aux, offset=12, ap=[[0, 1], [1, 1]]),
    )

    # pixel grids generated on-device at t=0 (x is 0..W-1 on every core;
    # y is 0..YQ-1 plus the per-core offset y0 from aux)
    xg_i = sb.tile([128, W], mybir.dt.int32, tag="xg_i")
    nc.gpsimd.iota(xg_i[:, :], pattern=[[1, W]], base=0, channel_multiplier=0)
    xg = sb.tile([128, W], f32, tag="xg")
    nc.gpsimd.tensor_copy(xg[:, :], xg_i[:, :])
    yg_i = sb.tile([128, YQ], mybir.dt.int32, tag="yg_i")
    nc.gpsimd.iota(yg_i[:, :], pattern=[[1, YQ]], base=0, channel_multiplier=0)
    yg0 = sb.tile([128, YQ], f32, tag="yg0")
    nc.gpsimd.tensor_copy(yg0[:, :], yg_i[:, :])
    y0b = sb.tile([128, 1], f32, tag="y0b")
    nc.gpsimd.partition_broadcast(y0b[:, :], grow[:, :])
    yg = sb.tile([128, YQ], f32, tag="yg")
    nc.gpsimd.tensor_scalar_add(yg[:, :], yg0[:, :], y0b[:, :])

    # ---------------- projection (PE) ----------------
    uvz_ps = ps.tile([128, NCHUNK * 3], f32, tag="uvz_ps")
    for c in range(NCHUNK):
        nc.tensor.matmul(
            uvz_ps[:, c * 3 : (c + 1) * 3],
            lhsT=posTs[:, c * 128 : (c + 1) * 128],
            rhs=pmt[:, :],
            start=True,
            stop=True,
        )
    uvz = uvz_ps.rearrange("p (c k) -> p c k", k=3)

    # keep the PE p-state warm between the projection and render matmuls
    # (otherwise the renders start at the throttled clock); results unused
    warm_ps = ps.tile([128, 112], f32, tag="warm_ps")
    for i in range(20):
        nc.tensor.matmul(
            warm_ps[:, :],
            lhsT=posTs[:, 0:128],
            rhs=posTs[:, 0:112],
            start=True,
            stop=True,
        )

    # ---------------- per-gaussian prep (DVE) ----------------
    # sp first: it only needs the gp DMA (arrives before the projection),
    # so it clears the DVE queue before the proj-dependent ops
    # sp = 1 / scale -> exponent = -0.5 * ((x - mx) * sp)^2 (0.5 folded
    # into the Exp scale)
    sp = sb.tile([128, NCHUNK], f32, tag="sp")
    nc.vector.reciprocal(sp[:, :], gp[:, :, 0])
    rz = sb.tile([128, NCHUNK], f32, tag="rz")
    nc.vector.reciprocal(rz[:, :], uvz[:, :, 2])
    my = sb.tile([128, NCHUNK], f32, tag="my")
    nc.vector.tensor_mul(my[:, :], uvz[:, :, 1], rz[:, :])
    mx = sb.tile([128, NCHUNK], f32, tag="mx")
    nc.vector.tensor_mul(mx[:, :], uvz[:, :, 0], rz[:, :])

    # ---------------- separable factors ----------------
    # y-side first: it feeds the matmul lhsT path (exp -> q*Ey on GPSIMD)
    ty = sb.tile([128, NCHUNK, YQ], f32, tag="ty")
    for c in range(NCHUNK):
        nc.vector.tensor_scalar(
            ty[:, c, :], yg[:, :], my[:, c : c + 1], sp[:, c : c + 1],
            op0=AT.subtract, op1=AT.mult,
        )
    ey = sb.tile([128, NCHUNK, YQ], f32, tag="ey")
    expy_is = []
    for h in range(2):
        hc = NCHUNK // 2
        yflat = ty[:, h * hc : (h + 1) * hc, :].rearrange("p c y -> p (c y)")
        eyflat = ey[:, h * hc : (h + 1) * hc, :].rearrange("p c y -> p (c y)")
        nc.scalar.activation(eyflat, yflat, AF.Square)
        expy_is.append(nc.scalar.activation(eyflat, eyflat, AF.Exp, scale=-0.5))
    expy_i = expy_is[-1]

    # x-side: fused affine on DVE, square on DVE (keeps ACT = exps only),
    # exp on ACT in groups so the PE can start on the first chunks
    tx = sb.tile([128, NCHUNK, W], f32, tag="tx")
    mm_dt = f32r if RENDER_MODE == "f32r" else f32
    ex = sb.tile([128, NCHUNK, W], mm_dt, tag="ex")
    for c0, c1 in ((0, 2), (2, 4), (4, 6), (6, 8)):
        for c in range(c0, c1):
            nc.vector.tensor_scalar(
                tx[:, c, :], xg[:, :], mx[:, c : c + 1], sp[:, c : c + 1],
                op0=AT.subtract, op1=AT.mult,
            )
        txh = tx[:, c0:c1, :].rearrange("p c x -> p (c x)")
        exh = ex[:, c0:c1, :].rearrange("p c x -> p (c x)")
        nc.vector.tensor_mul(exh, txh, txh)
        expx_i = nc.scalar.activation(exh, exh, AF.Exp, scale=-0.5)
        bass_rust.add_dep_helper(expx_i.ins, expy_i.ins, sync=False,
                                 reason="Exp_y feeds the PE-critical lhsT path")

    # channel weights (emitted late: only needed by wmat).
    # f32r: [opac, opac*r, opac*g, opac*b]
    # fp32: [opac, opac, opac*r, opac*g, opac*b] (den twice so channel
    # pairs stack to M=128 with den available on both partition halves)
    NQ = 5
    qw = sb.tile([128, NCHUNK, NQ], f32, tag="qw")
    for j in range(NQ - 3):
        nc.gpsimd.tensor_copy(qw[:, :, j], gp[:, :, 1])
    for ch in range(3):
        nc.gpsimd.tensor_mul(
            qw[:, :, NQ - 3 + ch], gp[:, :, 1], gp[:, :, ch + 2]
        )

    # ---------------- channel-scaled Ey (GPSIMD) ----------------
    # wmat[:, c, j, :] = qw[:, c, j] * ey[:, c, :]  via broadcast reads
    wmat = sb.tile([128, NCHUNK, NQ, YQ], mm_dt, tag="wmat")

    def wmat_op(eng, c0, c1):
        eyc = ey[:, c0:c1, :]
        ey_b = bass.AP(
            tensor=ey.tensor, offset=eyc.offset,
            ap=[eyc.ap[0], eyc.ap[1], [0, NQ], eyc.ap[2]],
        )
        qwc = qw[:, c0:c1, :]
        q_b = bass.AP(
            tensor=qw.tensor, offset=qwc.offset,
            ap=[qwc.ap[0], qwc.ap[1], qwc.ap[2], [0, YQ]],
        )
        eng.tensor_mul(wmat[:, c0:c1, :, :], ey_b, q_b)

    wmat_op(nc.gpsimd, 0, 1)
    wmat_op(nc.gpsimd, 1, 3)
    wmat_op(nc.gpsimd, 3, 5)
    wmat_op(nc.vector, 5, 8)

    # ---------------- render matmuls (PE, float32r) ----------------
    # ---------------- render matmuls + epilogue ----------------
    # Channel pairs stacked along the output partition dim (M=128) to
    # halve the streamed column count: acc_dd = [den | den], acc_rg =
    # [red | grn], acc_b = [blu] (M=64).  fp32 runs the PE's exact 2-pass
    # mode; f32r runs single-pass (~4x faster, ~8e-4 max rel err).
    acc_dd = ps.tile([128, W], f32, tag="acc_dd")
    acc_rg = ps.tile([128, W], f32, tag="acc_rg")
    acc_b = ps.tile([YQ, W], f32, tag="acc_b")
    # (chain, chunk) issue order: interleaved per chunk so each ex[c] is
    # consumed as it lands, EXCEPT the last two den matmuls are hoisted
    # ahead of the last color matmuls so the eps+reciprocal chain overlaps
    # the tail of the render stream.
    sched = [(c, j) for c in range(NCHUNK - 2) for j in range(3)]
    sched += [(6, 0), (7, 0), (6, 1), (7, 1), (6, 2), (7, 2)]
    chain_args = [
        lambda c: dict(lhsT=wmat[:, c, 0:2, :], out=acc_dd[:, :]),
        lambda c: dict(lhsT=wmat[:, c, 2:4, :], out=acc_rg[:, :]),
        lambda c: dict(lhsT=wmat[:, c, 4, :], out=acc_b[:, :]),
    ]
    for c, j in sched:
        a = chain_args[j](c)
        nc.tensor.matmul(
            a["out"], lhsT=a["lhsT"], rhs=ex[:, c, :],
            start=(c == 0), stop=(c == NCHUNK - 1),
        )
    rden = sb.tile([128, W], f32, tag="rden")
    nc.vector.tensor_scalar_add(rden[:, :], acc_dd[:, :], EPS)
    nc.vector.reciprocal(rden[:, :], rden[:, :])
    outt_rg = sb.tile([128, W], f32, tag="outt_rg")
    nc.vector.tensor_mul(outt_rg[:, :], acc_rg[:, :], rden[:, :])
    outt_b = sb.tile([YQ, W], f32, tag="outt_b")
    nc.vector.tensor_mul(outt_b[:, :], acc_b[:, :], rden[0:YQ, :])
    # rg on the ACT HWDGE queue, b (the critical tail transfer: it
    # follows the very last matmul) on the SP queue whose DMA-start
    # delay is 134ns shorter
    nc.scalar.dma_start(
        out=out[0:2, :, :].rearrange("c y x -> (c y) x"), in_=outt_rg[:, :]
    )
    nc.sync.dma_start(out=out[2, :, :], in_=outt_b[:, :])


def _build_module():
    nc = bacc.Bacc("TRN2", target_bir_lowering=False, debug=False)
    with tile.TileContext(nc) as tc:
        with (
            tc.tile_pool(name="sb", bufs=1) as sb,
            tc.tile_pool(name="ps", bufs=1, space="PSUM") as ps,
        ):
            build_kernel(nc, sb, ps)
    nc.compile()
    return nc


def _host_pm(qvec_b: np.ndarray, tvec_b: np.ndarray) -> np.ndarray:
    """Combined projection matrix: [x y z 1] @ pm = (u, v, z_cam) with
    proj = (u/z_cam, v/z_cam).  Mirrors reference._quat_to_rot."""
    q = qvec_b.astype(np.float64)
    q = q / np.linalg.norm(q)
    w_, x, y, z = q
    R = np.array(
        [
            [1 - 2 * (y * y + z * z), 2 * (x * y - z * w_), 2 * (x * z + y * w_)],
            [2 * (x * y + z * w_), 1 - 2 * (x * x + z * z), 2 * (y * z - x * w_)],
            [2 * (x * z - y * w_), 2 * (y * z + x * w_), 1 - 2 * (x * x + y * y)],
        ]
    )
    t = tvec_b.astype(np.float64)
    rows = np.stack([FX * R[0] + CX * R[2], FY * R[1] + CY * R[2], R[2]], axis=1)
    last = np.array([FX * t[0] + CX * t[2], FY * t[1] + CY * t[2], t[2]])
    return np.concatenate([rows, last[None, :]], axis=0).astype(np.float32)


def kernel(positions, colors, opacities, scales, qvec, tvec, pixel_coords):
    global _CACHED_NC, LAST_RESULTS
    if _CACHED_NC is None:
        _CACHED_NC = _build_module()
    nc = _CACHED_NC

    f32 = np.float32
    pos = np.asarray(positions, f32)
    colv = np.asarray(colors, f32)
    opv = np.asarray(opacities, f32).reshape(N)
    scv = np.asarray(scales, f32).reshape(N)
    pc = np.asarray(pixel_coords, f32).reshape(H, W, 2)
    xs = np.ascontiguousarray(pc[0, :, 0].reshape(1, W))
    ys_full = pc[:, 0, 1]

    # host-side packing (layout only; all math stays on device)
    posT_h = np.ascontiguousarray(
        np.concatenate([pos.T, np.ones((1, N), f32)], axis=0)
    )  # [4, N]
    gprops_h = np.ascontiguousarray(
        np.concatenate([scv.reshape(N, 1), opv.reshape(N, 1), colv], axis=1)
        .reshape(NCHUNK, 128, 5)
        .transpose(1, 0, 2)
    )  # [128, NCHUNK, 5]

    pms = [
        _host_pm(np.asarray(qvec, f32)[b], np.asarray(tvec, f32)[b]) for b in range(B)
    ]

    in_maps = []
    for core in range(NCORES):
        b, qy = divmod(core, 4)
        aux = np.concatenate(
            [pms[b].reshape(-1), ys_full[qy * YQ : qy * YQ + 1]]
        ).reshape(1, 13)
        in_maps.append(
            dict(
                post=posT_h,
                gprops=gprops_h,
                aux=np.ascontiguousarray(aux, dtype=f32),
            )
        )

    def _run_and_gather():
        res = run_bass_kernel_spmd(
            nc, in_maps, core_ids=list(range(NCORES)), trace=TRACE
        )
        outv = np.zeros((B, 3, H, W), f32)
        for core in range(NCORES):
            b, qy = divmod(core, 4)
            outv[b, :, qy * YQ : (qy + 1) * YQ, :] = np.asarray(
                res.results[core]["out"]
            )
        return res, outv

    # retries: the axon-proxied execute occasionally fails with a
    # transient NRT_EXEC_UNIT_UNRECOVERABLE worker error (can surface
    # lazily at result-fetch time) that clears on a later attempt
    last_exc = None
    for _attempt in range(3):
        try:
            res, outv = _run_and_gather()
            break
        except Exception as e:  # noqa: BLE001
            last_exc = e
    else:
        raise last_exc
    LAST_RESULTS = res
    return outv

